# revision 2
# baseline (speedup 1.0000x reference)
"""Physics-Attention (structured 3D mesh) — 8-core trn2 kernel, v4.

v4 = v3 (int8 wire + per-batch pipeline) with the host side tuned for the
1-CPU container: preallocated buffers, fewer numpy passes, D2H issued the
moment each batch is dispatched, per-shard fetches without a host concat.

Batch b runs on cores (2b, 2b+1); the slice-pooling psum stays inside the
pair.  Wire format: per-row int8 x (+f16 scales), int8 output (+f32 scale
per core).
"""

import numpy as np

B, N, DIM = 4, 65536, 64
HEADS, DH = 8, 32
INNER = HEADS * DH
SLICES = 64
GD, GH, GW = 32, 32, 32
NB = GD * GH * GW
NPP = GH * GW
NU = (N - NB) // 2
NS = NB // 2

_CACHE = {}


def _build():
    if "fns" in _CACHE:
        return _CACHE["fns"]
    import jax
    import jax.numpy as jnp
    from jax import lax

    bf = jnp.bfloat16

    NSLAB = 18 * 34 * 34     # 20808 padded slab points

    def core_fn(qcat, scat,
                temperature, fx_conv_w, fx_conv_b, fx_lin_w, fx_lin_b,
                xp_conv_w, xp_conv_b, xp_lin_w, xp_lin_b,
                slice_w, slice_b, wq, wk, wv, out_w, out_b):
        # qcat: [NSLAB + NU, 64] int8 (zero-padded slab then unstructured)
        # scat: [NSLAB + NU, 1] f16 per-row dequant scales
        xall = (qcat.astype(jnp.float32) * scat.astype(jnp.float32)).astype(bf)
        slab = xall[:NSLAB].reshape(18, 34, 34, DIM)
        xu = xall[NSLAB:]

        n_loc = NS + NU

        def project(cw, cb, lw, lb):
            out = jnp.zeros((16, GH, GW, INNER), jnp.float32)
            cwb = cw.astype(bf)
            for dz in range(3):
                for dy in range(3):
                    for dx in range(3):
                        patch = slab[dz:dz + 16, dy:dy + GH, dx:dx + GW, :]
                        out = out + jnp.einsum(
                            "zyxc,oc->zyxo", patch, cwb[:, :, dz, dy, dx],
                            preferred_element_type=jnp.float32)
            out = out + cb
            xb = out.reshape(NS, INNER)
            xe = xu @ lw.T.astype(bf) + lb
            return jnp.concatenate([xb, xe.astype(jnp.float32)], axis=0)

        fx = project(fx_conv_w, fx_conv_b, fx_lin_w, fx_lin_b)
        xm = project(xp_conv_w, xp_conv_b, xp_lin_w, xp_lin_b)
        fx = fx.reshape(n_loc, HEADS, DH)
        xm = xm.reshape(n_loc, HEADS, DH)

        temp = jnp.clip(temperature, 0.1, 5.0).reshape(1, HEADS, 1)
        logits = jnp.einsum("nhc,gc->nhg", xm, slice_w,
                            preferred_element_type=jnp.float32) + slice_b
        p = jax.nn.softmax(logits / temp, axis=-1)        # [n, h, g]

        norm_part = p.sum(axis=0)                         # [h, g]
        tok_part = jnp.einsum("nhc,nhg->hgc", fx.astype(bf), p.astype(bf),
                              preferred_element_type=jnp.float32)
        norm = lax.psum(norm_part, "i")
        tok = lax.psum(tok_part, "i")
        tok = tok / (norm + 1e-5)[..., None]              # [h, g, c]

        q = tok @ wq.T
        k = tok @ wk.T
        v = tok @ wv.T
        attn = jax.nn.softmax(
            jnp.einsum("hgc,hkc->hgk", q, k) * (DH ** -0.5), axis=-1)
        os_ = attn @ v                                    # [h, g, c]

        out_x = jnp.einsum("hgc,nhg->nhc", os_.astype(bf), p.astype(bf),
                           preferred_element_type=jnp.float32)
        out_x = out_x.reshape(n_loc, INNER)
        res = out_x.astype(bf) @ out_w.T.astype(bf) + out_b   # [n_loc, 64]

        oscale = jnp.abs(res).max() / 127.0
        oq = jnp.clip(jnp.round(res / oscale), -127, 127).astype(jnp.int8)
        return oq, oscale.astype(jnp.float32)

    devs = jax.devices()[:8]
    fns = [jax.pmap(core_fn, axis_name="i", in_axes=0,
                    devices=[devs[2 * b], devs[2 * b + 1]])
           for b in range(B)]
    _CACHE["fns"] = fns
    return fns


NSLAB = 18 * 34 * 34         # 20808 padded slab points
NCAT = NSLAB + NU            # 37192 wire points per core


def _buffers():
    if "bufs" in _CACHE:
        return _CACHE["bufs"]
    bufs = []
    for b in range(B):
        bufs.append(dict(
            qf=np.empty((N, DIM), np.float32),
            q=np.empty((N, DIM), np.int8),
            qcat=np.zeros((2, NCAT, DIM), np.int8),
            scat=np.zeros((2, NCAT, 1), np.float16),
        ))
    _CACHE["bufs"] = bufs
    return bufs


def _quant_shard(xb, bb):
    """Quantize one batch into bb's preallocated packed buffers."""
    rm = np.maximum(xb.max(axis=-1), -xb.min(axis=-1))[:, None]
    np.maximum(rm, np.float32(1e-5), out=rm)
    s = rm.astype(np.float16) * np.float16(1 / 127.0)     # wire scale [N,1]
    inv = np.float32(1.0) / s.astype(np.float32)
    qf, q = bb["qf"], bb["q"]
    np.multiply(xb, inv, out=qf)
    np.rint(qf, out=q, casting="unsafe")

    qcat, scat = bb["qcat"], bb["scat"]
    qg = q[:NB].reshape(GD, GH, GW, DIM)
    sg = s[:NB].reshape(GD, GH, GW, 1)
    for h in range(2):
        qslab = qcat[h, :NSLAB].reshape(18, 34, 34, DIM)   # contiguous view
        sslab = scat[h, :NSLAB].reshape(18, 34, 34, 1)
        lo, hi = 16 * h - 1, 16 * h + 17
        glo, ghi = max(lo, 0), min(hi, GD)
        qslab[glo - lo:ghi - lo, 1:33, 1:33] = qg[glo:ghi]
        sslab[glo - lo:ghi - lo, 1:33, 1:33] = sg[glo:ghi]
        qcat[h, NSLAB:] = q[NB + NU * h:NB + NU * (h + 1)]
        scat[h, NSLAB:] = s[NB + NU * h:NB + NU * (h + 1)]
    return qcat, scat


def kernel(x, temperature, fx_conv_w, fx_conv_b, fx_lin_w, fx_lin_b,
           xp_conv_w, xp_conv_b, xp_lin_w, xp_lin_b,
           slice_w, slice_b, wq, wk, wv, out_w, out_b):
    import jax
    fns = _build()
    bufs = _buffers()

    params = (temperature, fx_conv_w, fx_conv_b, fx_lin_w, fx_lin_b,
              xp_conv_w, xp_conv_b, xp_lin_w, xp_lin_b,
              slice_w, slice_b, wq, wk, wv, out_w, out_b)
    fp = sum(float(np.asarray(p).sum()) for p in params)
    if _CACHE.get("fp") != fp:
        devs = jax.devices()[:8]
        _CACHE["args"] = [
            [jax.device_put_replicated(np.asarray(p, dtype=np.float32),
                                       [devs[2 * b], devs[2 * b + 1]])
             for p in params]
            for b in range(B)]
        _CACHE["fp"] = fp
    args = _CACHE["args"]

    x = np.asarray(x)
    if x.dtype != np.float32:
        x = x.astype(np.float32)

    futs = []
    for b in range(B):
        qcat, scat = _quant_shard(x[b], bufs[b])
        oq, osc = fns[b](qcat, scat, *args[b])
        try:
            oq.copy_to_host_async()
            osc.copy_to_host_async()
        except Exception:
            pass
        futs.append((oq, osc))

    out = np.empty((B, N, DIM), dtype=np.float32)
    for b in range(B):
        oq, osc = futs[b]
        sh = [np.asarray(s.data) for s in oq.addressable_shards]
        sc = np.asarray(osc)
        d0 = sh[0].reshape(-1, DIM) if sh[0].ndim == 3 else sh[0]
        d1 = sh[1].reshape(-1, DIM) if sh[1].ndim == 3 else sh[1]
        np.multiply(d0[:NS], sc[0], out=out[b, 0:NS], dtype=np.float32)
        np.multiply(d1[:NS], sc[1], out=out[b, NS:NB], dtype=np.float32)
        np.multiply(d0[NS:], sc[0], out=out[b, NB:NB + NU], dtype=np.float32)
        np.multiply(d1[NS:], sc[1], out=out[b, NB + NU:N], dtype=np.float32)
    return out
